# revision 1
# baseline (speedup 1.0000x reference)
"""CRF loss (nn_EntityModel_crf) Bass/Tile kernel for Trainium2, 8 NeuronCores.

Strategy: data-parallel over batch (8 examples per core).  Per core:
  feat^T = W_out^T @ hidden^T   (PE transpose of hidden tiles + fp32r matmuls,
                                 W_out replicated 4x along its output dim so all
                                 128 PE rows are used)
  forward algorithm: reformulated as exp-space matrix-product chains.
    Each example's 511 recurrence steps are split into 8 chunks of 64 steps;
    each chunk is a product of T x T matrices  D_f * E^T  (E = exp(transitions),
    D_f = diag(exp(feat_s + b_out - SHIFT))).  The 64 chains (8 ex x 8 chunks)
    are stacked 4-per-partition-block so one scan round is a single
    [128,512] fp32r matmul with block-diag(E) + one DVE broadcast-multiply.
    The constant SHIFT subtracted per step inside exp keeps magnitudes bounded
    (no renormalization); sent_score = ln(Z) + SHIFT*S at the end.
  gold score: one-hot(tags) built with is_equal(tags, iota); emission and
    transition gathers are fused multiply+accumulate ops plus one small matmul.
  combine: per-example chain of 8 tiny matmuls a <- W^T a, then Z = 1^T a.

kernel(**inputs) takes the FULL inputs, shards on host, runs the module on
cores 0-7 via run_bass_kernel_spmd, and sums the per-example losses.
"""

import numpy as np

import concourse.bass as bass
import concourse.tile as tile
from concourse import mybir
from concourse._compat import with_exitstack
from concourse.bass_utils import run_bass_kernel_spmd

B, S, H, T = 64, 512, 768, 32
NCORES = 8
BL = B // NCORES          # 8 examples per core
BS = BL * S               # 4096 (bs = b_local*512 + s)
NBC = BS // 256           # 16 big chunks of 256 bs-columns
SHIFT = 4.125             # per-step shift inside exp, for fp32 range safety
                          # (actual mean log-growth/step is ~4.115 for these
                          # input distributions; 4.125 is exact in fp32)
CORR = SHIFT * S          # added back to ln(Z)

F32 = mybir.dt.float32
F32R = mybir.dt.float32r
AF = mybir.ActivationFunctionType
ALU = mybir.AluOpType


@with_exitstack
def _crf_kernel(ctx, tc, loss8, ins):
    nc = tc.nc
    f = F32

    # ---------------- persistent SBUF ----------------
    consts = ctx.enter_context(tc.tile_pool(name="consts", bufs=1))
    persist = ctx.enter_context(tc.tile_pool(name="persist", bufs=1))
    vp = ctx.enter_context(tc.tile_pool(name="vp", bufs=2))
    apl = ctx.enter_context(tc.tile_pool(name="apl", bufs=2))
    a4p = ctx.enter_context(tc.tile_pool(name="a4p", bufs=2))

    def cload(name, shape, dt=F32):
        t = consts.tile(list(shape), dt, tag=name)
        nc.sync.dma_start(t[:], ins[name])
        return t

    w4_t = consts.tile([128, 6 * 128], F32R, tag="w4")
    nc.sync.dma_start(
        w4_t[:].rearrange("p (k m) -> p k m", k=6),
        ins["w4"].rearrange("(k p) m -> p k m", p=128),
    )
    ident_t = cload("ident", (128, 128))
    eyeT_t = cload("eyeT", (128, 512))
    bsprd_t = cload("bsprd4", (T, 4 * 128), F32R)
    a4i_t = cload("a4init", (128, BL), F32R)
    iota_t = cload("iota", (T, 1))
    ones_t = cload("ones", (128, 8), F32R)
    transT_t = cload("transT", (T, T), F32R)
    trans4_t = cload("trans4", (128, T))
    bout4_t = cload("bout4", (128, 1))
    tags_t = cload("tags32", (T, BS))

    ef4 = persist.tile([128, 1024], f, tag="ef4")      # exp(feat+b-SHIFT), chain layout
    featl = persist.tile([T, BS], f, tag="featl")      # feat + b_out, [t, bs]
    u0buf = persist.tile([T, BL], f, tag="u0buf")      # exp(feat_0+b-SHIFT) per example
    E4 = persist.tile([128, 128], F32R, tag="E4")         # block-diag exp(transitions)
    bm4 = persist.tile([128, 1], f, tag="bm4")         # b_out - SHIFT (4x replicated)
    onehot = persist.tile([T, BS], F32R, tag="onehot")
    Ared = persist.tile([T, BL], f, tag="Ared")
    Rred = persist.tile([T, BL], f, tag="Rred")
    gsum = persist.tile([T, BL], F32R, tag="gsum")
    snap = persist.tile([128, 256], F32R, tag="snap")     # chain-7 matrices at round 62
    sent = persist.tile([1, BL], f, tag="sent")
    lossv = persist.tile([1, BL], f, tag="lossv")

    # Warm up the ACT table set (natural_log_exp) on a dummy op with a single
    # wait: walrus attaches the ACT_TABLE_LOAD to the first Exp/Ln activation,
    # and that site cannot carry many sync waits.
    dummy = persist.tile([1, 2], f, tag="dummy")
    nc.vector.memset(dummy[:], 0.0)
    nc.scalar.activation(dummy[:, 0:1], dummy[:, 1:2], AF.Exp)

    # E4 = block-diag(exp(transitions)); bm4 = b_out - SHIFT
    # trans4 is bounced through DVE so the Exp ops wait on a single (DVE)
    # semaphore: ACT instructions cannot carry more than one sync wait.
    trc = persist.tile([128, T], f, tag="trc")
    nc.vector.tensor_copy(trc[:], trans4_t[:])
    nc.sync.dma_start(E4[:], ins["zeros"][:, 0:128])
    # snap rows 0:96 stay zero; contraction over them must contribute nothing
    nc.sync.dma_start(snap[:], ins["zeros"])
    for c in range(4):
        sl = slice(32 * c, 32 * c + 32)
        nc.scalar.activation(E4[sl, sl], trc[sl, :], AF.Exp)
    nc.vector.tensor_scalar(bm4[:], bout4_t[:], -SHIFT, None, op0=ALU.add)

    # one-hot of gold tags: onehot[t, bs] = (tags[bs] == t); masked tags are T
    nc.vector.tensor_scalar(onehot[:], tags_t[:], iota_t[:], None, op0=ALU.is_equal)

    # chain-7 (c4=3, odd c16) col j=63 is a dummy step whose result is
    # discarded; fill with ones so no uninitialized/NaN data is read.  Done on
    # ACT so the ef4 exp writes (also ACT) need no cross-engine wait for it.
    nc.scalar.copy(ef4[96:128, 127::128], ones_t[96:128, 0:8])

    # ---------------- feat phase ----------------
    # big chunk bc covers bs in [bc*256, bc*256+256); example b = bc//2.
    # psum rows are 4 identical copies of feat (W tiled 4x); row-block q takes
    # chain k = q + 4*(bc%2): ef4[32q+t', 64*bc+j] = exp(feat[t', s=256bc+64q+1+j]+b-SHIFT)
    with (
        tc.tile_pool(name="hidp", bufs=3) as hidp,
        tc.tile_pool(name="xtp", bufs=2) as xtp,
        tc.tile_pool(name="scr", bufs=2) as scr,
        tc.tile_pool(name="pst", bufs=4, space="PSUM") as pst,
        tc.tile_pool(name="psf", bufs=2, space="PSUM") as psf,
        tc.tile_pool(name="psq", bufs=1, space="PSUM") as psqp,
    ):
        for bc in range(NBC):
            hid_t = hidp.tile([128, 1536], f, tag="hid")
            nc.sync.dma_start(
                hid_t[:].rearrange("p (c h) -> p c h", c=2),
                ins["hid"][bc * 256 : (bc + 1) * 256, :].rearrange(
                    "(c p) h -> p c h", p=128
                ),
            )
            ps_f = psf.tile([128, 256], f, tag="psf")
            for k in range(6):
                xt = xtp.tile([128, 256], F32R, tag=f"xt{k}")
                ps_t = pst.tile([128, 256], f, tag="pst")
                for c in range(2):
                    col0 = c * 768 + 128 * k
                    nc.tensor.transpose(
                        ps_t[:, c * 128 : (c + 1) * 128],
                        hid_t[:, col0 : col0 + 128],
                        ident_t[:],
                    )
                nc.scalar.copy(xt[:], ps_t[:])
                nc.tensor.matmul(
                    ps_f[:],
                    w4_t[:, k * 128 : (k + 1) * 128],
                    xt[:],
                    start=(k == 0),
                    stop=(k == 5),
                )
            # raw feat (+b_out) for the gold emission gather
            nc.vector.tensor_scalar(
                featl[:, bc * 256 : (bc + 1) * 256],
                ps_f[0:32, :],
                bout4_t[0:32, :],
                None,
                op0=ALU.add,
            )
            # exp(feat + b_out - SHIFT) into chain layout
            for q in range(4):
                n = 64 if q < 3 else 63
                rs = slice(32 * q, 32 * q + 32)
                nc.scalar.activation(
                    ef4[rs, bc * 64 : bc * 64 + n],
                    ps_f[rs, 64 * q + 1 : 64 * q + 1 + n],
                    AF.Exp,
                    bias=bm4[rs, :],
                )
            if bc % 2 == 1:
                # col j=63 of chain (c4=3, c16=bc-1): step s_local=256 lives in
                # this (odd) big-chunk's psum col 0
                nc.scalar.activation(
                    ef4[96:128, (bc - 1) * 64 + 63 : (bc - 1) * 64 + 64],
                    ps_f[96:128, 0:1],
                    AF.Exp,
                    bias=bm4[96:128, :],
                )
            else:
                b = bc // 2
                nc.scalar.activation(
                    u0buf[:, b : b + 1], ps_f[0:32, 0:1], AF.Exp, bias=bm4[0:32, :]
                )
            if bc % 2 == 1:
                # gold gathers for example b (featl rows for b complete now)
                b = bc // 2
                oh = onehot[:, b * S : (b + 1) * S]
                sA = scr.tile([T, S], f, tag="scrA")
                nc.vector.scalar_tensor_tensor(
                    sA[:],
                    oh,
                    0.0,
                    featl[:, b * S : (b + 1) * S],
                    op0=ALU.add,
                    op1=ALU.mult,
                    accum_out=Ared[:, b : b + 1],
                )
                psq = psqp.tile([T, S], f, tag="psq")
                nc.tensor.matmul(
                    psq[:],
                    transT_t[:],
                    oh,
                    start=True,
                    stop=True,
                )
                sR = scr.tile([T, S - 1], f, tag="scrR")
                nc.vector.scalar_tensor_tensor(
                    sR[:],
                    onehot[:, b * S : b * S + S - 1],
                    0.0,
                    psq[0:32, 1:S],
                    op0=ALU.add,
                    op1=ALU.mult,
                    accum_out=Rred[:, b : b + 1],
                )

    # ---------------- scan phase ----------------
    # v[c4*32+t', c16*32+t] : chain (c4, c16) state matrix W[t', t].
    # round j: W <- (E^T W) * f_col;  f_col = ef4[:, c16*64 + j] broadcast over t
    with (
        tc.tile_pool(name="pss", bufs=4, space="PSUM") as pss,
        tc.tile_pool(name="psA", bufs=1, space="PSUM") as psAp,
        tc.tile_pool(name="psB", bufs=1, space="PSUM") as psBp,
        tc.tile_pool(name="psZ", bufs=1, space="PSUM") as psZp,
        tc.tile_pool(name="psG", bufs=1, space="PSUM") as psGp,
    ):
        v = vp.tile([128, 512], F32R, tag="v")
        nc.vector.tensor_copy(v[:], eyeT_t[:])
        # chain k=0 (c4=0, even c16) starts as diag(u0)
        ev = eyeT_t[0:32, :].rearrange("p (b two t) -> p b two t", two=2, t=32)[
            :, :, 0, :
        ]
        vv = v[0:32, :].rearrange("p (b two t) -> p b two t", two=2, t=32)[:, :, 0, :]
        u0b = u0buf[:, :].unsqueeze(2).broadcast_to([T, BL, T])
        nc.vector.tensor_tensor(vv, ev, u0b, op=ALU.mult)

        v62 = None
        for j in range(64):
            vn = vp.tile([128, 512], F32R, tag="v")
            for h in range(2):
                cs = slice(h * 256, (h + 1) * 256)
                ps = pss.tile([128, 256], f, tag="pss")
                nc.tensor.matmul(
                    ps[:],
                    E4[:],
                    v[:, cs],
                    start=True,
                    stop=True,
                )
                fcol = (
                    ef4[:, h * 512 + j : h * 512 + 512 : 64]
                    .unsqueeze(2)
                    .broadcast_to([128, 8, T])
                )
                nc.vector.tensor_tensor(
                    vn[:, cs].rearrange("p (c t) -> p c t", t=32),
                    ps[:].rearrange("p (c t) -> p c t", t=32),
                    fcol,
                    op=ALU.mult,
                )
            v = vn
            if j == 62:
                v62 = v
        # chain 7 finished its 63 real steps at round 62; snapshot before the
        # dummy round-63 write lands in those blocks.
        sv = v62[96:128, :].rearrange("p (m two t) -> p m two t", two=2, t=32)[
            :, :, 1, :
        ]
        nc.vector.tensor_copy(snap[96:128, :].rearrange("p (m t) -> p m t", t=32), sv)

        # ---------------- combine + final ----------------
        # a' = W^T a per example, as full-height [K=128,M=32,N=8] fp32r
        # matmuls: rhs is a replicated into row-block c4 and zero elsewhere,
        # so the other three stacked chain-blocks contribute nothing.
        # (walrus fp32r matmuls require base partition 0 and N >= 2.)
        a4m = a4i_t  # ones in rows 96:128 (c4 = 3 = stage 7), zeros elsewhere
        aS = None
        for i in range(7, -1, -1):
            c4 = i % 4
            psS = psAp.tile([T, 8 * BL], f, tag="psA")
            for b in range(BL):
                c16 = 2 * b + i // 4
                if i == 7:
                    lh = snap[:, b * 32 : (b + 1) * 32]
                else:
                    lh = v[:, c16 * 32 : (c16 + 1) * 32]
                nc.tensor.matmul(
                    psS[:, 8 * b : 8 * b + 8],
                    lh,
                    a4m[:, 0:BL],
                    start=True,
                    stop=True,
                )
            aS = apl.tile([T, BL], F32R, tag="aS")
            nc.vector.tensor_copy(aS[:], psS[:, 0 : 8 * BL : 9])
            if i > 0:
                c4n = (i - 1) % 4
                psB = psBp.tile([128, BL], f, tag="psB")
                nc.tensor.matmul(
                    psB[:],
                    bsprd_t[:, c4n * 128 : (c4n + 1) * 128],
                    aS[:],
                    start=True,
                    stop=True,
                )
                a4n = a4p.tile([128, BL], F32R, tag="a4")
                nc.vector.tensor_copy(a4n[:], psB[:])
                a4m = a4n
        psZ = psZp.tile([1, BL], f, tag="psZ")
        nc.tensor.matmul(
            psZ[:],
            ones_t[0:32, 0:1],
            aS[:],
            start=True,
            stop=True,
        )
        # Ln table-set swap happens on this low-wait dummy (reads psZ so it
        # schedules here, after every Exp), not on the real Ln below.
        nc.scalar.activation(dummy[:, 1:2], psZ[0:1, 0:1], AF.Ln)
        nc.scalar.activation(sent[:], psZ[:], AF.Ln)
        nc.vector.tensor_add(gsum[:], Ared[:], Rred[:])
        psG = psGp.tile([1, BL], f, tag="psG")
        nc.tensor.matmul(
            psG[:],
            ones_t[0:32, 0:1],
            gsum[:],
            start=True,
            stop=True,
        )
        # loss[b] = (sent + CORR) - gold
        nc.vector.scalar_tensor_tensor(
            lossv[:], sent[:], CORR, psG[0:1, :], op0=ALU.add, op1=ALU.subtract
        )
        nc.sync.dma_start(loss8, lossv[:])


# Instruction types whose multi-wait handling walrus supports natively (DMA
# descriptors / drain use a different wait mechanism).  Everything else gets
# at most one sync wait per instruction; extras move to same-engine NoOps.
_MULTIWAIT_OK = {
    "InstAllEngineBarrier",
    "InstEventSemaphore",
}


def _split_sync_waits(nc):
    """neuronxcc/walrus codegen accepts only one sync-wait command per compute
    instruction (the Tile native backend supports several).  Hoist extra waits
    onto NoOps in front of the instruction on the same engine queue."""
    nid = [0]
    for fn in nc.m.functions:
        for blk in fn.blocks:
            out = []
            changed = False
            for inst in blk.instructions:
                si = inst.sync_info
                waits = list(si.on_wait) if si and si.on_wait else []
                if len(waits) > 1 and type(inst).__name__ not in _MULTIWAIT_OK:
                    changed = True
                    for w in waits[:-1]:
                        nop = mybir.InstNoOp(name=f"I-wsplit-{nid[0]}")
                        nid[0] += 1
                        nop.engine = inst.engine
                        nop.sync_info = mybir.SyncInfo(on_wait=[w], on_update=[])
                        out.append(nop)
                    inst.sync_info = mybir.SyncInfo(
                        on_wait=[waits[-1]], on_update=list(si.on_update or [])
                    )
                out.append(inst)
            if changed:
                blk.set_instructions(out) if hasattr(blk, "set_instructions") else None
                try:
                    blk.instructions = out
                except Exception:
                    del blk.instructions[:]
                    blk.instructions.extend(out)


_NC_CACHE = []


def build_module(for_hw=True, repeat=1):
    nc = bass.Bass(
        "TRN2", target_bir_lowering=False, debug=False, num_devices=NCORES
    )
    shapes = {
        "hid": (BS, H),
        "w4": (H, 128),
        "transT": (T, T),
        "trans4": (128, T),
        "bout4": (128, 1),
        "tags32": (T, BS),
        "iota": (T, 1),
        "ones": (128, 8),
        "eyeT": (128, 512),
        "ident": (128, 128),
        "bsprd4": (T, 4 * 128),
        "a4init": (128, BL),
        "zeros": (128, 256),
    }
    r_names = {"w4", "transT", "ones", "bsprd4", "a4init", "zeros"}
    ins = {
        name: nc.dram_tensor(
            name, list(shape), F32R if name in r_names else F32, kind="ExternalInput"
        ).ap()
        for name, shape in shapes.items()
    }
    out = nc.dram_tensor("loss8", [1, BL], F32, kind="ExternalOutput").ap()
    with tile.TileContext(nc) as tc:
        if repeat > 1:
            with tc.For_i(0, repeat, 1):
                _crf_kernel(tc, out, ins)
        else:
            _crf_kernel(tc, out, ins)
    if for_hw:
        # only needed for the neuronxcc/walrus path; CoreSim rejects the NoOps
        _split_sync_waits(nc)
    return nc


def _bsprd4():
    # bsprd4[t, c4*128 + p] = 1 iff p == c4*32 + t  (replicate-into-block-c4)
    m = np.zeros((T, 4 * 128), dtype=np.float32)
    for c4 in range(4):
        m[:, c4 * 128 + c4 * 32 : c4 * 128 + (c4 + 1) * 32] = np.eye(T)
    return m


def _a4init():
    m = np.zeros((128, BL), dtype=np.float32)
    m[96:128, :] = 1.0
    return m


def make_in_maps(hidden, mask, target_tag, W_out, b_out, transitions):
    hidden = np.ascontiguousarray(hidden, dtype=np.float32)
    mask = np.asarray(mask)
    tags = np.where(mask != 0, target_tag, T).astype(np.float32)  # [B, S]
    shared = {
        "w4": np.ascontiguousarray(np.tile(np.asarray(W_out, np.float32), (1, 4))),
        "transT": np.ascontiguousarray(np.asarray(transitions, np.float32).T),
        "trans4": np.ascontiguousarray(
            np.tile(np.asarray(transitions, np.float32), (4, 1))
        ),
        "bout4": np.ascontiguousarray(
            np.tile(np.asarray(b_out, np.float32), 4)[:, None]
        ),
        "iota": np.arange(T, dtype=np.float32)[:, None],
        "ones": np.ones((128, 8), dtype=np.float32),
        "eyeT": np.ascontiguousarray(
            np.tile(np.eye(T, dtype=np.float32), (4, 16))
        ),
        "ident": np.eye(128, dtype=np.float32),
        "bsprd4": _bsprd4(),
        "a4init": _a4init(),
        "zeros": np.zeros((128, 256), dtype=np.float32),
    }
    in_maps = []
    for c in range(NCORES):
        hid = hidden[c * BL : (c + 1) * BL].reshape(BS, H)
        tg = tags[c * BL : (c + 1) * BL].reshape(1, BS)
        m = dict(shared)
        m["hid"] = np.ascontiguousarray(hid)
        m["tags32"] = np.ascontiguousarray(np.broadcast_to(tg, (T, BS)))
        in_maps.append(m)
    return in_maps


def kernel(hidden, mask, target_tag, W_out, b_out, transitions):
    if not _NC_CACHE:
        _NC_CACHE.append(build_module())
    nc = _NC_CACHE[0]
    in_maps = make_in_maps(hidden, mask, target_tag, W_out, b_out, transitions)
    res = run_bass_kernel_spmd(nc, in_maps, core_ids=list(range(NCORES)))
    total = 0.0
    for r in res.results:
        total += float(np.sum(np.asarray(r["loss8"], dtype=np.float64)))
    return np.float32(total)



# revision 21
# speedup vs baseline: 1.0235x; 1.0235x over previous
"""CRF loss (nn_EntityModel_crf) Bass/Tile kernel for Trainium2, 8 NeuronCores.

Strategy: data-parallel over batch (8 examples per core).  Per core:
  feat^T = W_out^T @ hidden^T   (PE transpose of hidden tiles + fp32r matmuls,
                                 W_out replicated 4x along its output dim so all
                                 128 PE rows are used)
  forward algorithm: reformulated as exp-space matrix-product chains.
    Each example's 511 recurrence steps are split into 8 chunks of 64 steps;
    each chunk is a product of T x T matrices  D_f * E^T  (E = exp(transitions),
    D_f = diag(exp(feat_s + b_out - SHIFT))).  The 64 chains (8 ex x 8 chunks)
    are stacked 4-per-partition-block so one scan round is a single
    [128,512] fp32r matmul with block-diag(E) + one DVE broadcast-multiply.
    The constant SHIFT subtracted per step inside exp keeps magnitudes bounded
    (no renormalization); sent_score = ln(Z) + SHIFT*S at the end.
  gold score: one-hot(tags) built with is_equal(tags, iota); emission and
    transition gathers are fused multiply+accumulate ops plus one small matmul.
  combine: per-example chain of 8 tiny matmuls a <- W^T a, then Z = 1^T a.

kernel(**inputs) takes the FULL inputs, shards on host, runs the module on
cores 0-7 via run_bass_kernel_spmd, and sums the per-example losses.
"""

import numpy as np

import concourse.bass as bass
import concourse.tile as tile
from concourse import mybir
from concourse._compat import with_exitstack
from concourse.bass_utils import run_bass_kernel_spmd

B, S, H, T = 64, 512, 768, 32
NCORES = 8
BL = B // NCORES          # 8 examples per core
BS = BL * S               # 4096 (bs = b_local*512 + s)
NBC = BS // 256           # 16 big chunks of 256 bs-columns
DVE_C16 = 10              # scan multiply split: c16 chains on DVE vs Pool
SHIFT = 4.125             # per-step shift inside exp, for fp32 range safety
                          # (actual mean log-growth/step is ~4.115 for these
                          # input distributions; 4.125 is exact in fp32)
CORR = SHIFT * S          # added back to ln(Z)

F32 = mybir.dt.float32
F32R = mybir.dt.float32r
BF16 = mybir.dt.bfloat16
AF = mybir.ActivationFunctionType
ALU = mybir.AluOpType


@with_exitstack
def _crf_kernel(ctx, tc, loss8, ins):
    nc = tc.nc
    f = F32

    # ---------------- persistent SBUF ----------------
    consts = ctx.enter_context(tc.tile_pool(name="consts", bufs=1))
    persist = ctx.enter_context(tc.tile_pool(name="persist", bufs=1))
    vp = ctx.enter_context(tc.tile_pool(name="vp", bufs=2))
    apl = ctx.enter_context(tc.tile_pool(name="apl", bufs=2))
    a4p = ctx.enter_context(tc.tile_pool(name="a4p", bufs=2))

    def cload(name, shape, dt=F32):
        t = consts.tile(list(shape), dt, tag=name)
        nc.sync.dma_start(t[:], ins[name])
        return t

    w4_t = consts.tile([128, 6 * 128], BF16, tag="w4")
    nc.sync.dma_start(
        w4_t[:].rearrange("p (k m) -> p k m", k=6),
        ins["w4"].rearrange("(k p) m -> p k m", p=128),
    )
    eyeT_t = cload("eyeT", (128, 512))
    bsprd_t = cload("bsprd4", (T, 4 * 128), F32R)
    a4i_t = cload("a4init", (128, BL), F32R)
    iota_t = cload("iota", (T, 1))
    ones_t = cload("ones", (128, 8), F32R)
    transT_t = cload("transT", (T, T), F32R)
    trans4_t = cload("trans4", (128, T))
    bout4_t = cload("bout4", (128, 1))
    tags_t = cload("tags32", (T, BS))

    ef4 = persist.tile([128, 1024], f, tag="ef4")      # exp(feat+b-SHIFT), chain layout
    featl = persist.tile([T, BS], f, tag="featl")      # feat + b_out, [t, bs]
    u0buf = persist.tile([T, BL], f, tag="u0buf")      # exp(feat_0+b-SHIFT) per example
    E4 = persist.tile([128, 128], F32R, tag="E4")         # block-diag exp(transitions)
    bm4 = persist.tile([128, 1], f, tag="bm4")         # b_out - SHIFT (4x replicated)
    onehot = persist.tile([T, BS], F32R, tag="onehot")
    Ared = persist.tile([T, BL], f, tag="Ared")
    Rred = persist.tile([T, BL], f, tag="Rred")
    gsum = persist.tile([T, BL], F32R, tag="gsum")
    snap = persist.tile([128, 256], F32R, tag="snap")     # chain-7 matrices at round 62
    sent = persist.tile([1, BL], f, tag="sent")
    lossv = persist.tile([1, BL], f, tag="lossv")

    # Warm up the ACT table set (natural_log_exp) on a dummy op with a single
    # wait: walrus attaches the ACT_TABLE_LOAD to the first Exp/Ln activation,
    # and that site cannot carry many sync waits.
    dummy = persist.tile([1, 2], f, tag="dummy")
    nc.vector.memset(dummy[:], 0.0)
    nc.scalar.activation(dummy[:, 0:1], dummy[:, 1:2], AF.Exp)

    # E4 = block-diag(exp(transitions)); bm4 = b_out - SHIFT
    # trans4 is bounced through DVE so the Exp ops wait on a single (DVE)
    # semaphore: ACT instructions cannot carry more than one sync wait.
    trc = persist.tile([128, T], f, tag="trc")
    nc.vector.tensor_copy(trc[:], trans4_t[:])
    nc.sync.dma_start(E4[:], ins["zeros"][:, 0:128])
    # snap rows 0:96 stay zero; contraction over them must contribute nothing
    nc.sync.dma_start(snap[:], ins["zeros"])
    for c in range(4):
        sl = slice(32 * c, 32 * c + 32)
        nc.scalar.activation(E4[sl, sl], trc[sl, :], AF.Exp)
    nc.vector.tensor_scalar(bm4[:], bout4_t[:], -SHIFT, None, op0=ALU.add)

    # one-hot of gold tags: onehot[t, bs] = (tags[bs] == t); masked tags are T
    nc.vector.tensor_scalar(onehot[:], tags_t[:], iota_t[:], None, op0=ALU.is_equal)

    # chain-7 (c4=3, odd c16) col j=63 is a dummy step whose result is
    # discarded; fill with ones so no uninitialized/NaN data is read.  Done on
    # ACT so the ef4 exp writes (also ACT) need no cross-engine wait for it.
    nc.scalar.copy(ef4[96:128, 127::128], ones_t[96:128, 0:8])

    # ---------------- feat phase ----------------
    # hidT (host-transposed, bf16) arrives as 6 row-tiles of [128, BS]; each
    # DMA'd in column halves so feat chunks overlap the tail of the stream.
    # big chunk bc covers bs in [bc*256, bc*256+256); example b = bc//2.
    # psum rows are 4 identical copies of feat (W tiled 4x); row-block q takes
    # chain k = q + 4*(bc%2): ef4[32q+t', 64*bc+j] = exp(feat[t', s=256bc+64q+1+j]+b-SHIFT)
    hidt = persist.tile([128, 6 * BS], BF16, tag="hidt")
    for quarter in range(4):
        h0, h1 = quarter * (BS // 4), (quarter + 1) * (BS // 4)
        for k in range(6):
            nc.sync.dma_start(
                hidt[:, k * BS + h0 : k * BS + h1],
                ins["hidT"][k * 128 : (k + 1) * 128, h0:h1],
            )
    with (
        tc.tile_pool(name="scr", bufs=2) as scr,
        tc.tile_pool(name="psf", bufs=4, space="PSUM") as psf,
        tc.tile_pool(name="psq", bufs=1, space="PSUM") as psqp,
    ):
        for bc in range(NBC):
            c0 = bc * 256
            ps_f = psf.tile([128, 256], f, tag="psf")
            for k in range(6):
                nc.tensor.matmul(
                    ps_f[:],
                    w4_t[:, k * 128 : (k + 1) * 128],
                    hidt[:, k * BS + c0 : k * BS + c0 + 256],
                    start=(k == 0),
                    stop=(k == 5),
                )
            # raw feat (+b_out) for the gold emission gather
            nc.vector.tensor_scalar(
                featl[:, bc * 256 : (bc + 1) * 256],
                ps_f[0:32, :],
                bout4_t[0:32, :],
                None,
                op0=ALU.add,
            )
            # exp(feat + b_out - SHIFT) into chain layout
            for q in range(4):
                n = 64 if q < 3 else 63
                rs = slice(32 * q, 32 * q + 32)
                nc.scalar.activation(
                    ef4[rs, bc * 64 : bc * 64 + n],
                    ps_f[rs, 64 * q + 1 : 64 * q + 1 + n],
                    AF.Exp,
                    bias=bm4[rs, :],
                )
            if bc % 2 == 1:
                # col j=63 of chain (c4=3, c16=bc-1): step s_local=256 lives in
                # this (odd) big-chunk's psum col 0
                nc.scalar.activation(
                    ef4[96:128, (bc - 1) * 64 + 63 : (bc - 1) * 64 + 64],
                    ps_f[96:128, 0:1],
                    AF.Exp,
                    bias=bm4[96:128, :],
                )
            else:
                b = bc // 2
                nc.scalar.activation(
                    u0buf[:, b : b + 1], ps_f[0:32, 0:1], AF.Exp, bias=bm4[0:32, :]
                )
            if bc % 2 == 1:
                # gold gathers for example b (featl rows for b complete now)
                b = bc // 2
                oh = onehot[:, b * S : (b + 1) * S]
                sA = scr.tile([T, S], f, tag="scrA")
                nc.vector.scalar_tensor_tensor(
                    sA[:],
                    oh,
                    0.0,
                    featl[:, b * S : (b + 1) * S],
                    op0=ALU.add,
                    op1=ALU.mult,
                    accum_out=Ared[:, b : b + 1],
                )
                psq = psqp.tile([T, S], f, tag="psq")
                nc.tensor.matmul(
                    psq[:],
                    transT_t[:],
                    oh,
                    start=True,
                    stop=True,
                )
                sR = scr.tile([T, S - 1], f, tag="scrR")
                nc.vector.scalar_tensor_tensor(
                    sR[:],
                    onehot[:, b * S : b * S + S - 1],
                    0.0,
                    psq[0:32, 1:S],
                    op0=ALU.add,
                    op1=ALU.mult,
                    accum_out=Rred[:, b : b + 1],
                )

    # ---------------- scan phase ----------------
    # v[c4*32+t', c16*32+t] : chain (c4, c16) state matrix W[t', t].
    # round j: W <- (E^T W) * f_col;  f_col = ef4[:, c16*64 + j] broadcast over t
    with (
        tc.tile_pool(name="pss", bufs=2, space="PSUM") as pss,
        tc.tile_pool(name="psb", bufs=2, space="PSUM") as psb,
        tc.tile_pool(name="psA", bufs=1, space="PSUM") as psAp,
        tc.tile_pool(name="psB", bufs=1, space="PSUM") as psBp,
        tc.tile_pool(name="psZ", bufs=1, space="PSUM") as psZp,
        tc.tile_pool(name="psG", bufs=1, space="PSUM") as psGp,
    ):
        v = vp.tile([128, 512], F32R, tag="v")
        nc.vector.tensor_copy(v[:], eyeT_t[:])
        # chain k=0 (c4=0, even c16) starts as diag(u0)
        ev = eyeT_t[0:32, :].rearrange("p (b two t) -> p b two t", two=2, t=32)[
            :, :, 0, :
        ]
        vv = v[0:32, :].rearrange("p (b two t) -> p b two t", two=2, t=32)[:, :, 0, :]
        u0b = u0buf[:, :].unsqueeze(2).broadcast_to([T, BL, T])
        nc.vector.tensor_tensor(vv, ev, u0b, op=ALU.mult)

        v62 = None
        for j in range(64):
            vn = vp.tile([128, 512], F32R, tag="v")
            for h in range(2):
                cs = slice(h * 256, (h + 1) * 256)
                ps = (pss if h == 0 else psb).tile([128, 256], f, tag="pss")
                nc.tensor.matmul(ps[:], E4[:], v[:, cs], start=True, stop=True)
                fcol = (
                    ef4[:, h * 512 + j : h * 512 + 512 : 64]
                    .unsqueeze(2)
                    .broadcast_to([128, 8, T])
                )
                nc.vector.tensor_tensor(
                    vn[:, cs].rearrange("p (c t) -> p c t", t=32),
                    ps[:].rearrange("p (c t) -> p c t", t=32),
                    fcol,
                    op=ALU.mult,
                )
            v = vn
            if j == 62:
                v62 = v
        # chain 7 finished its 63 real steps at round 62; snapshot before the
        # dummy round-63 write lands in those blocks.
        sv = v62[96:128, :].rearrange("p (m two t) -> p m two t", two=2, t=32)[
            :, :, 1, :
        ]
        nc.vector.tensor_copy(snap[96:128, :].rearrange("p (m t) -> p m t", t=32), sv)

        # ---------------- combine + final ----------------
        # a' = W^T a per example, as full-height [K=128,M=32,N=8] fp32r
        # matmuls: rhs is a replicated into row-block c4 and zero elsewhere,
        # so the other three stacked chain-blocks contribute nothing.
        # (walrus fp32r matmuls require base partition 0 and N >= 2.)
        a4m = a4i_t  # ones in rows 96:128 (c4 = 3 = stage 7), zeros elsewhere
        aS = None
        for i in range(7, -1, -1):
            c4 = i % 4
            psS = psAp.tile([T, 8 * BL], f, tag="psA")
            for b in range(BL):
                c16 = 2 * b + i // 4
                if i == 7:
                    lh = snap[:, b * 32 : (b + 1) * 32]
                else:
                    lh = v[:, c16 * 32 : (c16 + 1) * 32]
                nc.tensor.matmul(
                    psS[:, 8 * b : 8 * b + 8],
                    lh,
                    a4m[:, 0:BL],
                    start=True,
                    stop=True,
                )
            aS = apl.tile([T, BL], F32R, tag="aS")
            nc.vector.tensor_copy(aS[:], psS[:, 0 : 8 * BL : 9])
            if i > 0:
                c4n = (i - 1) % 4
                psB = psBp.tile([128, BL], f, tag="psB")
                nc.tensor.matmul(
                    psB[:],
                    bsprd_t[:, c4n * 128 : (c4n + 1) * 128],
                    aS[:],
                    start=True,
                    stop=True,
                )
                a4n = a4p.tile([128, BL], F32R, tag="a4")
                nc.vector.tensor_copy(a4n[:], psB[:])
                a4m = a4n
        psZ = psZp.tile([1, BL], f, tag="psZ")
        nc.tensor.matmul(
            psZ[:],
            ones_t[0:32, 0:1],
            aS[:],
            start=True,
            stop=True,
        )
        # Ln table-set swap happens on this low-wait dummy (reads psZ so it
        # schedules here, after every Exp), not on the real Ln below.
        nc.scalar.activation(dummy[:, 1:2], psZ[0:1, 0:1], AF.Ln)
        nc.scalar.activation(sent[:], psZ[:], AF.Ln)
        nc.vector.tensor_add(gsum[:], Ared[:], Rred[:])
        psG = psGp.tile([1, BL], f, tag="psG")
        nc.tensor.matmul(
            psG[:],
            ones_t[0:32, 0:1],
            gsum[:],
            start=True,
            stop=True,
        )
        # loss[b] = (sent + CORR) - gold
        nc.vector.scalar_tensor_tensor(
            lossv[:], sent[:], CORR, psG[0:1, :], op0=ALU.add, op1=ALU.subtract
        )
        nc.sync.dma_start(loss8, lossv[:])


# Instruction types whose multi-wait handling walrus supports natively (DMA
# descriptors / drain use a different wait mechanism).  Everything else gets
# at most one sync wait per instruction; extras move to same-engine NoOps.
_MULTIWAIT_OK = {
    "InstAllEngineBarrier",
    "InstEventSemaphore",
}


def _split_sync_waits(nc):
    """neuronxcc/walrus codegen accepts only one sync-wait command per compute
    instruction (the Tile native backend supports several).  Hoist extra waits
    onto NoOps in front of the instruction on the same engine queue."""
    nid = [0]
    for fn in nc.m.functions:
        for blk in fn.blocks:
            out = []
            changed = False
            for inst in blk.instructions:
                si = inst.sync_info
                waits = list(si.on_wait) if si and si.on_wait else []
                if len(waits) > 1 and type(inst).__name__ not in _MULTIWAIT_OK:
                    changed = True
                    for w in waits[:-1]:
                        nop = mybir.InstNoOp(name=f"I-wsplit-{nid[0]}")
                        nid[0] += 1
                        nop.engine = inst.engine
                        nop.sync_info = mybir.SyncInfo(on_wait=[w], on_update=[])
                        out.append(nop)
                    inst.sync_info = mybir.SyncInfo(
                        on_wait=[waits[-1]], on_update=list(si.on_update or [])
                    )
                out.append(inst)
            if changed:
                blk.set_instructions(out) if hasattr(blk, "set_instructions") else None
                try:
                    blk.instructions = out
                except Exception:
                    del blk.instructions[:]
                    blk.instructions.extend(out)


_NC_CACHE = []


def build_module(for_hw=True, repeat=1):
    nc = bass.Bass(
        "TRN2", target_bir_lowering=False, debug=False, num_devices=NCORES
    )
    shapes = {
        "hidT": (H, BS),
        "w4": (H, 128),
        "transT": (T, T),
        "trans4": (128, T),
        "bout4": (128, 1),
        "tags32": (T, BS),
        "iota": (T, 1),
        "ones": (128, 8),
        "eyeT": (128, 512),
        "bsprd4": (T, 4 * 128),
        "a4init": (128, BL),
        "zeros": (128, 256),
    }
    r_names = {"transT", "ones", "bsprd4", "a4init", "zeros"}
    dts = {"hidT": BF16, "w4": BF16}
    ins = {
        name: nc.dram_tensor(
            name,
            list(shape),
            dts.get(name, F32R if name in r_names else F32),
            kind="ExternalInput",
        ).ap()
        for name, shape in shapes.items()
    }
    out = nc.dram_tensor("loss8", [1, BL], F32, kind="ExternalOutput").ap()
    with tile.TileContext(nc) as tc:
        if repeat > 1:
            with tc.For_i(0, repeat, 1):
                _crf_kernel(tc, out, ins)
        else:
            _crf_kernel(tc, out, ins)
    if for_hw:
        # only needed for the neuronxcc/walrus path; CoreSim rejects the NoOps
        _split_sync_waits(nc)
    return nc


def _bsprd4():
    # bsprd4[t, c4*128 + p] = 1 iff p == c4*32 + t  (replicate-into-block-c4)
    m = np.zeros((T, 4 * 128), dtype=np.float32)
    for c4 in range(4):
        m[:, c4 * 128 + c4 * 32 : c4 * 128 + (c4 + 1) * 32] = np.eye(T)
    return m


def _a4init():
    m = np.zeros((128, BL), dtype=np.float32)
    m[96:128, :] = 1.0
    return m


def make_in_maps(hidden, mask, target_tag, W_out, b_out, transitions):
    import ml_dtypes

    bf16 = ml_dtypes.bfloat16
    # host-side transpose to [H, B*S] so the kernel needs no PE transposes
    hidT = np.asarray(hidden, np.float32).transpose(2, 0, 1).reshape(H, B * S)
    hidT = np.ascontiguousarray(hidT).astype(bf16)
    mask = np.asarray(mask)
    tags = np.where(mask != 0, target_tag, T).astype(np.float32)  # [B, S]
    shared = {
        "w4": np.ascontiguousarray(
            np.tile(np.asarray(W_out, np.float32), (1, 4))
        ).astype(bf16),
        "transT": np.ascontiguousarray(np.asarray(transitions, np.float32).T),
        "trans4": np.ascontiguousarray(
            np.tile(np.asarray(transitions, np.float32), (4, 1))
        ),
        "bout4": np.ascontiguousarray(
            np.tile(np.asarray(b_out, np.float32), 4)[:, None]
        ),
        "iota": np.arange(T, dtype=np.float32)[:, None],
        "ones": np.ones((128, 8), dtype=np.float32),
        "eyeT": np.ascontiguousarray(
            np.tile(np.eye(T, dtype=np.float32), (4, 16))
        ),
        "bsprd4": _bsprd4(),
        "a4init": _a4init(),
        "zeros": np.zeros((128, 256), dtype=np.float32),
    }
    in_maps = []
    for c in range(NCORES):
        tg = tags[c * BL : (c + 1) * BL].reshape(1, BS)
        m = dict(shared)
        m["hidT"] = np.ascontiguousarray(hidT[:, c * BS : (c + 1) * BS])
        m["tags32"] = np.ascontiguousarray(np.broadcast_to(tg, (T, BS)))
        in_maps.append(m)
    return in_maps


def kernel(hidden, mask, target_tag, W_out, b_out, transitions):
    if not _NC_CACHE:
        _NC_CACHE.append(build_module())
    nc = _NC_CACHE[0]
    in_maps = make_in_maps(hidden, mask, target_tag, W_out, b_out, transitions)
    res = run_bass_kernel_spmd(nc, in_maps, core_ids=list(range(NCORES)))
    total = 0.0
    for r in res.results:
        total += float(np.sum(np.asarray(r["loss8"], dtype=np.float64)))
    return np.float32(total)

